# revision 1
# baseline (speedup 1.0000x reference)
"""NeighborSample Trainium2 kernel, v16: i=0 as DRAM->DRAM on the SWDGE ring.

Input  x:   (8, 64, 64, 192) f32
Output:     (8*64*64, 5, 5, 192) f32 — out[b*4096 + h*64 + w, i, j, c] =
            x[b, h+i-2, w+j-2, c] (zero-padded).

Pure DMA, data-parallel over batch (1 sample per NeuronCore). Input is
zero-padded on the host to (68, 68, 192).

Two independent HWDGE rings (sync / scalar), one per h-half; half's 36
padded rows live in partitions 64*half .. +35 so each half's store reads
hit a disjoint 8-port group of the SBUF AXI fabric.

DMA fan-out rule (measured): outermost count c is split over n =
(largest divisor of c <= 16) engine slots, c/n consecutive rows per
slot; prime c lands on ONE engine. Loads on a ring are ack-serialized at
~7.9 GB/s per engine, so a 36-row load takes ~20 us; splitting it as
c=32 + c=4 lets the i=0 store (which only needs rows 0..31) start at
~13 us. Separate semaphores per load: a wait can otherwise be satisfied
by the OTHER load's 16 increments (sem increments are not engine-exact
for sub-16-slot DMAs). Cross-DMA data dependencies always need a
semaphore gate — ring FIFO order does NOT give write-visibility
(measured: gateless load->store produces wrong data). The i>=1 gate
releases while i=0's 2048 descriptors still fill the ring, so the rings
never run dry.

Engine 15 measures ~10% slower, so ring A's i=4 is c=30 + c=2: slot 15
gets no rows in either (c=30 -> 15 slots x 2 rows; c=2 -> slots 0,1).
Total HBM traffic per core: 3.7 MB read + 78.6 MB write.
"""

import sys

for _p in ("/opt/trn_rl_repo",):
    if _p not in sys.path:
        sys.path.insert(0, _p)

import numpy as np

import concourse.bass as bass
import concourse.mybir as mybir
from concourse.bass_utils import run_bass_kernel_spmd

B = 8
H = W = 64
C = 192
K = 5
PAD = 2
HP = H + 2 * PAD     # 68 padded rows
WP = W + 2 * PAD     # 68 padded cols
ROW = WP * C         # 13056 elems per partition (one padded row)
WIN = K * C          # 960: one (h, w, i) output chunk
OUT_W = K * K * C    # 4800
OUT_H = W * OUT_W    # 307200
HH = H // 2          # 32 output rows per half
HPH = HH + 2 * PAD   # 36 padded rows per half


def _store(eng, out, buf, half, i, h0, cnt):
    """Store shift i for this half's local output rows [h0, h0+cnt)."""
    return eng.dma_start(
        out=bass.AP(
            out,
            (HH * half + h0) * OUT_H + i * WIN,
            [[OUT_H, cnt], [OUT_W, W], [1, WIN]],
        ),
        in_=bass.AP(
            buf, (64 * half + i + h0) * ROW, [[ROW, cnt], [C, W], [1, WIN]]
        ),
    )


def _load(eng, x, buf, half, r0, cnt):
    """Load this half's padded rows [r0, r0+cnt) into partitions."""
    return eng.dma_start(
        out=bass.AP(buf, (64 * half + r0) * ROW, [[ROW, cnt], [1, ROW]]),
        in_=bass.AP(x, (HH * half + r0) * ROW, [[ROW, cnt], [1, ROW]]),
    )


def _dram_store_i0(eng, x, out, half):
    """i=0 shift direct from the padded input in DRAM — no SBUF, no deps."""
    return eng.dma_start(
        out=bass.AP(
            out,
            (HH * half) * OUT_H + 0 * WIN,
            [[OUT_H, HH], [OUT_W, W], [1, WIN]],
        ),
        in_=bass.AP(x, (HH * half) * ROW, [[ROW, HH], [C, W], [1, WIN]]),
    )


def _emit(eng, x, out, buf, l1, l3, dsem, half, tail_split):
    # i=0 runs DRAM->DRAM on the gpsimd ring; rows 1..35 suffice here
    _load(eng, x, buf, half, 1, 32).then_inc(l1, 16)
    _load(eng, x, buf, half, 33, 3).then_inc(l3, 16)
    eng.wait_ge(l1, 16)
    n_stores = 0
    for i in range(1, K - 1):
        if i == 1:
            _store(eng, out, buf, half, i, 0, HH).then_inc(dsem, 16)
            n_stores += 1
            eng.wait_ge(l3, 16)
        else:
            _store(eng, out, buf, half, i, 0, HH).then_inc(dsem, 16)
            n_stores += 1
    if tail_split:
        _store(eng, out, buf, half, K - 1, 0, HH - 2).then_inc(dsem, 16)
        _store(eng, out, buf, half, K - 1, HH - 2, 2).then_inc(dsem, 16)
        n_stores += 2
    else:
        _store(eng, out, buf, half, K - 1, 0, HH).then_inc(dsem, 16)
        n_stores += 1
    eng.wait_ge(dsem, 16 * n_stores)


def build_nc() -> bass.Bass:
    nc = bass.Bass()
    x = nc.declare_dram_parameter("x", [HP, WP, C], mybir.dt.float32, isOutput=False)
    out = nc.declare_dram_parameter(
        "out", [H, W, K, K, C], mybir.dt.float32, isOutput=True
    )

    with (
        nc.Block() as block,
        nc.semaphore("l1_a") as l1_a,
        nc.semaphore("l3_a") as l3_a,
        nc.semaphore("d_a") as d_a,
        nc.semaphore("l1_b") as l1_b,
        nc.semaphore("l3_b") as l3_b,
        nc.semaphore("d_b") as d_b,
        nc.semaphore("d_g") as d_g,
        nc.sbuf_tensor("buf", [128, ROW], mybir.dt.float32) as buf,
    ):

        @block.sync
        def _(sync):
            _emit(sync, x, out, buf, l1_a, l3_a, d_a, 0, tail_split=True)

        @block.scalar
        def _(scalar):
            _emit(scalar, x, out, buf, l1_b, l3_b, d_b, 1, tail_split=False)

        @block.gpsimd
        def _(gpsimd):
            _dram_store_i0(gpsimd, x, out, 0).then_inc(d_g, 16)
            _dram_store_i0(gpsimd, x, out, 1).then_inc(d_g, 16)
            gpsimd.wait_ge(d_g, 32)

    return nc


_NC_CACHE = None


def prep_in_maps(x):
    xp = np.zeros((B, HP, WP, C), dtype=np.float32)
    xp[:, PAD : PAD + H, PAD : PAD + W, :] = x
    return [{"x": np.ascontiguousarray(xp[i])} for i in range(B)]


def kernel(x) -> np.ndarray:
    global _NC_CACHE
    x = np.asarray(x, dtype=np.float32)
    assert x.shape == (B, H, W, C), x.shape
    if _NC_CACHE is None:
        _NC_CACHE = build_nc()
    in_maps = prep_in_maps(x)
    res = run_bass_kernel_spmd(_NC_CACHE, in_maps, list(range(B)))
    outs = [res.results[i]["out"].reshape(H * W, K, K, C) for i in range(B)]
    return np.concatenate(outs, axis=0)

